# revision 16
# baseline (speedup 1.0000x reference)
"""Multi-head attention (B=2, S=2048, D=1024, H=16) on 8 Trainium2 NeuronCores.

Sharding: 2-way data parallel over batch x 4-way tensor parallel over heads.
Core c -> batch c//4, head group c%4 (4 heads = 256 features per core).

v2 schedule: the ScalarE exp stream (16.8M exps @ ~1 elem/lane/cycle) is the
critical path; everything is organized to start it early and never stall it.

  - x/weights bf16; out-proj (xh, wo) also bf16 (rel err stays ~5e-3)
  - host pre-swizzles all DRAM tensors so every DMA descriptor is >=1KB
    contiguous per partition (input stream was descriptor-bound before)
  - minimal first DMA wave (wk, xk chunk0 halves, wq, xq chunk0 halves) so
    the first score matmuls + exp start at ~8us (was 36.7us); wv/xv0/xv1 go
    on the Activation engine's hw DGE queue, which is idle pre-exp
  - PSUM layout (8 banks): exp ring A=[128,4,512] (4) + B=[128,2,512] (2),
    alternating so ScalarE always has a full group ready; 2 x [128,512]
    scratch slots (S) rotate PV bursts / projections / out-proj
  - scores transposed, two heads of a pair on disjoint PE row groups
    (partition offsets 0-63 / 64-127) so both stream concurrently
  - exp group sizes: qc0 uses 1 key-tile groups (proj work needs the
    scratch slots early), qc1-3 use 2+1 mixed groups (fewer, bigger
    ACTIVATEs: ~18us/pair exp stream)
  - PV accumulates per (pair, half, head) in short 8-matmul bursts through
    a 1-bank scratch slot (M=128: 64 v-dims + 64 ones columns giving the
    softmax denominator for free); halves combined by a DVE add into SBUF
  - softmax normalize: DVE reciprocal_approx_fast (18 bits) + multiply
  - ScalarE does exp ONLY; all PSUM evacuation on VectorE
"""

import sys

for _p in ("/opt/trn_rl_repo", "/root/.axon_site/_ro/trn_rl_repo"):
    if _p not in sys.path:
        sys.path.insert(0, _p)

import numpy as np

P = 128
S = 2048          # sequence length (per batch)
DM = 1024         # model dim
DH = 256          # features per core (4 heads x 64)
NH = 4            # heads per core
DK = 64           # head dim
KT = DM // P      # 8 contraction tiles over model dim
NKT = S // P      # 16 key tiles
QC = 512          # query chunk (free dim of matmuls)
NQC = S // QC     # 4 query chunks
N_CORES = 8

PROFILE = False          # set True (module-level) to capture an NTFF trace
LAST_EXEC_NS = None      # filled when PROFILE is True and tracing succeeds
LAST_RESULTS = None      # BassKernelResults of the last profiled run

_NC_CACHE = {}


def _split_waits(nc, mybir, maxw=1):
    """This container's walrus accepts only one sync-wait command per
    instruction; hoist extra waits onto preceding NoOps on the same engine."""
    for f in nc.m.functions:
        for b in f.blocks:
            out = []
            changed = False
            for inst in list(b.instructions):
                si = getattr(inst, "sync_info", None)
                if si is not None and si.on_wait and len(si.on_wait) > maxw:
                    waits = list(si.on_wait)
                    extra, keep = waits[:-maxw], waits[-maxw:]
                    for j in range(0, len(extra), maxw):
                        out.append(mybir.InstNoOp(
                            name=f"{inst.name}-wsplit{j}",
                            engine=inst.engine,
                            sync_info=mybir.SyncInfo(
                                on_wait=list(extra[j:j + maxw]), on_update=[]),
                            bass_nofuse=True,
                        ))
                    si.on_wait = keep
                    changed = True
                out.append(inst)
            if changed:
                b.instructions = out


def _build_nc():
    import concourse.bass as bass
    import concourse.tile as tile
    import concourse.mybir as mybir

    f32 = mybir.dt.float32
    bf16 = mybir.dt.bfloat16
    Exp = mybir.ActivationFunctionType.Exp
    MUL = mybir.AluOpType.mult
    ADD = mybir.AluOpType.add

    nc = bass.Bass()

    # host-preswizzled layouts (see kernel()):
    #   x*  [P, NQC, KT, QC]  -- per partition, one 8KB contiguous chunk per qc
    #   w*  [P, KT, DH]       -- 4KB contiguous per partition
    #   wo  [P, 2, DM]        -- 4KB contiguous per partition
    #   outT[P, NQC, 8, QC]   -- out-proj writes 2KB contiguous per partition
    xq = nc.dram_tensor("xq", [P, NQC, KT, QC], bf16, kind="ExternalInput")
    xk = nc.dram_tensor("xk", [P, NQC, KT, QC], bf16, kind="ExternalInput")
    xv = nc.dram_tensor("xv", [P, NQC, KT, QC], bf16, kind="ExternalInput")
    wq = nc.dram_tensor("wq", [P, KT, DH], bf16, kind="ExternalInput")
    wk = nc.dram_tensor("wk", [P, KT, DH], bf16, kind="ExternalInput")
    wv = nc.dram_tensor("wv", [P, KT, DH], bf16, kind="ExternalInput")
    wo = nc.dram_tensor("wo", [P, 2, DM], bf16, kind="ExternalInput")
    outT = nc.dram_tensor("outT", [P, NQC, 8, QC], bf16, kind="ExternalOutput")

    with tile.TileContext(nc) as tc:
        with (
            tc.tile_pool(name="w", bufs=1) as wpool,
            tc.tile_pool(name="xc", bufs=1) as xcpool,
            tc.tile_pool(name="qk", bufs=1) as qkpool,
            tc.tile_pool(name="vp", bufs=1) as vpool,
            tc.tile_pool(name="xhp", bufs=1) as xhpool,
            tc.tile_pool(name="pp", bufs=1) as ppool,
            tc.tile_pool(name="ac", bufs=1) as acpool,
            tc.tile_pool(name="op", bufs=1) as opool,
            tc.tile_pool(name="ps", bufs=1, space="PSUM") as pspool,
        ):
            # ---- persistent SBUF tensors ----
            wq_sb = wpool.tile([P, KT, DH], bf16, tag="wq")
            wk_sb = wpool.tile([P, KT, DH], bf16, tag="wk")
            wv_sb = wpool.tile([P, KT, DH], bf16, tag="wv")
            wo_sb = wpool.tile([P, 2, DM], bf16, tag="wo")
            dumw = wpool.tile([P, QC], bf16, tag="dumw")
            nc.gpsimd.memset(dumw[:], 0.0)

            qT = qkpool.tile([P, 2, S], bf16, tag="qT")   # Q^T feature-major
            kT = qkpool.tile([P, 2, S], bf16, tag="kT")   # K^T feature-major
            # per (key-tile, head): [V_h (64 cols) | ones (64 cols)]: the PV
            # matmul then emits the softmax denominator on partitions 64-127
            v_sb = vpool.tile([P, NKT, NH, 2 * DK], bf16, tag="v")
            xh = xhpool.tile([P, 2, S], bf16, tag="xh")   # normalized attn out

            ones_f32 = wpool.tile([P, 1], f32, tag="ones")
            nc.vector.memset(ones_f32[:], 1.0)
            nc.vector.tensor_copy(
                v_sb[:, :, :, DK:2 * DK],
                ones_f32[:].to_broadcast([P, NKT, NH, DK]))

            # ---- DMA: Act hw-DGE queue carries the V prologue (idle pre-exp)
            cs_v = [None] * NQC
            cs_v[0] = xcpool.tile([P, KT, QC], bf16, tag="xv", bufs=4, name="cs_v0")
            cs_v[1] = xcpool.tile([P, KT, QC], bf16, tag="xv", bufs=4, name="cs_v1")
            _V_DMA_ENG = "scalar"  # Act hw-DGE queue: idle pre-exp
            _veng = getattr(nc, _V_DMA_ENG)
            _veng.dma_start(wv_sb[:], wv[:, :, :])
            _veng.dma_start(cs_v[0][:], xv[:, 0, :, :])
            _veng.dma_start(cs_v[1][:], xv[:, 1, :, :])

            # ---- DMA: SP queue, need-ordered; chunk0 x in halves so the
            # first projections start mid-transfer
            cs_k = [None] * NQC
            cs_q = [None] * NQC
            nc.sync.dma_start(wk_sb[:], wk[:, :, :])
            cs_k[0] = xcpool.tile([P, KT, QC], bf16, tag="xk", bufs=4, name="cs_k0")
            for hh in range(2):
                nc.sync.dma_start(cs_k[0][:, 4 * hh:4 * hh + 4, :],
                                  xk[:, 0, 4 * hh:4 * hh + 4, :])
            nc.sync.dma_start(wq_sb[:], wq[:, :, :])
            cs_q[0] = xcpool.tile([P, KT, QC], bf16, tag="xq", bufs=2, name="cs_q0")
            for hh in range(2):
                nc.sync.dma_start(cs_q[0][:, 4 * hh:4 * hh + 4, :],
                                  xq[:, 0, 4 * hh:4 * hh + 4, :])
            for ch in (1, 2, 3):
                cs_k[ch] = xcpool.tile([P, KT, QC], bf16, tag="xk",
                                       bufs=4, name=f"cs_k{ch}")
                nc.sync.dma_start(cs_k[ch][:], xk[:, ch, :, :])
            for ch in (2, 3):
                cs_v[ch] = xcpool.tile([P, KT, QC], bf16, tag="xv",
                                       bufs=4, name=f"cs_v{ch}")
                nc.sync.dma_start(cs_v[ch][:], xv[:, ch, :, :])
            cs_q[1] = xcpool.tile([P, KT, QC], bf16, tag="xq", bufs=2,
                                  name="cs_q1")
            nc.sync.dma_start(cs_q[1][:], xq[:, 1, :, :])
            nc.sync.dma_start(wo_sb[:], wo[:, :, :])
            for ch in (2, 3):
                cs_q[ch] = xcpool.tile([P, KT, QC], bf16, tag="xq",
                                       bufs=2, name=f"cs_q{ch}")
                nc.sync.dma_start(cs_q[ch][:], xq[:, ch, :, :])

            # ---- helpers ----
            def dummy_mms(n):
                """PE warm-up filler in the DMA-bound prologue (A/B slots)."""
                for i in range(n):
                    tg = "A" if i % 2 == 0 else "B"
                    ps = pspool.tile([P, QC], f32, tag=tg)
                    nc.tensor.matmul(ps[:], dumw[:, 0:P], dumw[:],
                                     start=True, stop=True)

            def proj_k(ch, pt):
                ps = pspool.tile([P, QC], f32, tag="S", bufs=2)
                for kt in range(KT):
                    nc.tensor.matmul(
                        ps[:], wk_sb[:, kt, pt * P:(pt + 1) * P],
                        cs_k[ch][:, kt, :],
                        start=(kt == 0), stop=(kt == KT - 1))
                nc.vector.tensor_copy(kT[:, pt, ch * QC:(ch + 1) * QC], ps[:])

            def proj_q(qc, pt):
                ps = pspool.tile([P, QC], f32, tag="S", bufs=2)
                for kt in range(KT):
                    nc.tensor.matmul(
                        ps[:], wq_sb[:, kt, pt * P:(pt + 1) * P],
                        cs_q[qc][:, kt, :],
                        start=(kt == 0), stop=(kt == KT - 1))
                nc.vector.tensor_copy(qT[:, pt, qc * QC:(qc + 1) * QC], ps[:])

            def proj_v(vt):
                """Project v key-tiles (2*vt, 2*vt+1): token-major."""
                ch = vt // 2
                ps = pspool.tile([P, QC], f32, tag="S", bufs=2)
                for t in range(2):
                    j = (2 * vt + t) % 4
                    for kt in range(KT):
                        nc.tensor.matmul(
                            ps[:, t * DH:(t + 1) * DH],
                            cs_v[ch][:, kt, j * P:(j + 1) * P],
                            wv_sb[:, kt, :],
                            start=(kt == 0), stop=(kt == KT - 1))
                nc.vector.tensor_copy(
                    v_sb[:, 2 * vt:2 * vt + 2, :, 0:DK],
                    ps[:].rearrange("p (t h d) -> p t h d", t=2, h=NH))

            def scores_group(qc, pair, kt2s, tag, pm):
                """Scores + exp for a group of key-tiles; boosted priority so
                the scheduler keeps the exp stream fed ahead of PV/proj
                side-work emitted around it."""
                qsl = slice(qc * QC, (qc + 1) * QC)
                n = len(kt2s)
                shape = [P, 4, QC] if tag == "A" else [P, 2, QC]
                ps = pspool.tile(shape, f32, tag=tag)
                pt = ppool.tile([P, shape[1], QC], bf16, tag="p" + tag, bufs=6)
                for i, kt2 in enumerate(kt2s):
                    ksl = slice(kt2 * P, (kt2 + 1) * P)
                    nc.tensor.matmul(
                        ps[:, 2 * i, :], kT[0:DK, pair, ksl],
                        qT[0:DK, pair, qsl], start=True, stop=True)
                    nc.tensor.matmul(
                        ps[:, 2 * i + 1, :], kT[DK:P, pair, ksl],
                        qT[DK:P, pair, qsl], start=True, stop=True)
                    pm[kt2] = (pt, 2 * i)
                nc.scalar.activation(
                    pt[:, 0:2 * n, :], ps[:, 0:2 * n, :], Exp, scale=0.125)

            def pv_burst(pair, half, j, pm, xacc, xaccF):
                """PV+denominator for key-tiles half*8..half*8+7, head j.
                xacc/xaccF are bf16: the normalize chain then runs in the
                DVE 2-byte 2x mode (and rel err stays ~6e-3)."""
                ps = pspool.tile([P, QC], f32, tag="S", bufs=2)
                k0, k1 = half * 8, half * 8 + 7
                for kt2 in range(k0, k1 + 1):
                    t, base = pm[kt2]
                    nc.tensor.matmul(
                        ps[:], v_sb[:, kt2, 2 * pair + j, :],
                        t[:, base + j, :],
                        start=(kt2 == k0), stop=(kt2 == k1))
                if half == 0:
                    nc.vector.tensor_copy(xacc[:, j, :], ps[:])
                else:
                    nc.vector.tensor_tensor(
                        xaccF[:, j, :], ps[:], xacc[:, j, :], ADD)

            def normalize(qc, pair, xaccF):
                qsl = slice(qc * QC, (qc + 1) * QC)
                rec = acpool.tile([P, 2, QC], bf16, tag="rec", bufs=2)
                with nc.allow_low_precision(reason="18-bit recip is plenty"):
                    for j in range(2):
                        nc.vector.reciprocal(
                            rec[0:DK, j, :], xaccF[DK:P, j, :])
                        nc.vector.tensor_tensor(
                            xh[j * DK:(j + 1) * DK, pair, qsl],
                            xaccF[0:DK, j, :], rec[0:DK, j, :], MUL)

            def outproj2(qc, i, evac=None):
                """Out-projection for output row groups 2i, 2i+1 of chunk qc."""
                qsl = slice(qc * QC, (qc + 1) * QC)
                ot = opool.tile([P, 2, QC], bf16, tag="ot", bufs=3)
                for gi in range(2):
                    g = 2 * i + gi
                    ps = pspool.tile([P, QC], f32, tag="S", bufs=2)
                    for kt in range(2):
                        nc.tensor.matmul(
                            ps[:], wo_sb[:, kt, g * P:(g + 1) * P],
                            xh[:, kt, qsl], start=(kt == 0), stop=(kt == 1))
                    if evac == "scalar":
                        nc.scalar.copy(ot[:, gi, :], ps[:])
                    else:
                        nc.vector.tensor_copy(ot[:, gi, :], ps[:])
                nc.sync.dma_start(outT[:, qc, 2 * i:2 * i + 2, :], ot[:])

            # ---- prologue: PE filler + first projections ----
            dummy_mms(12)
            proj_k(0, 0)
            proj_k(0, 1)
            proj_q(0, 0)

            # ---- main loop ----
            # pairs alternate A-led / B-led so the first scores group of a
            # pair never waits on the previous pair's last exp (its slot)
            GROUPS_QC0 = [([k], "A" if k % 2 == 0 else "B")
                          for k in range(NKT)]
            GROUPS_MIX_A = [([0, 1], "A"), ([2], "B"), ([3, 4], "A"),
                            ([5], "B"), ([6, 7], "A"), ([8], "B"),
                            ([9, 10], "A"), ([11], "B"), ([12, 13], "A"),
                            ([14], "B"), ([15], "A")]
            GROUPS_MIX_B = [([0], "B"), ([1, 2], "A"), ([3], "B"),
                            ([4, 5], "A"), ([6], "B"), ([7, 8], "A"),
                            ([9], "B"), ([10, 11], "A"), ([12], "B"),
                            ([13, 14], "A"), ([15], "B")]

            # PVb + normalize of a pair run early in the NEXT pair (the PE
            # fills the boundary while ScalarE streams into fresh slots)
            deferred = []
            for qc in range(NQC):
                for pair in range(2):
                    xacc = acpool.tile([P, 2, QC], bf16, tag="xacc", bufs=2)
                    xaccF = acpool.tile([P, 2, QC], bf16, tag="xaccF", bufs=2)
                    pm = {}

                    def pva(j, pair=pair, pm=pm, xacc=xacc, xaccF=xaccF):
                        pv_burst(pair, 0, j, pm, xacc, xaccF)

                    if qc == 0:
                        groups = GROUPS_QC0
                        if pair == 0:
                            side = {
                                0: [lambda: proj_v(0)],
                                1: [lambda: proj_q(0, 1)],
                                2: [lambda: proj_v(1)],
                                3: [lambda: proj_k(1, 0)],
                                4: [lambda: proj_v(2)],
                                6: [lambda: proj_v(3)],
                                7: [lambda: proj_k(2, 0)],
                                8: [lambda: pva(0)],
                                9: [lambda: pva(1)],
                                11: [lambda: proj_k(3, 0)],
                                12: [lambda: proj_v(4)],
                                13: [lambda: proj_v(5)],
                            }
                        else:
                            side = {
                                0: [lambda: proj_v(6)],
                                1: [lambda: proj_v(7)],
                                2: [deferred[0]],
                                3: [deferred[1], lambda: proj_k(1, 1)],
                                4: [deferred[2]],
                                5: [lambda: proj_q(1, 0)],
                                7: [lambda: proj_k(2, 1)],
                                8: [lambda: pva(0)],
                                9: [lambda: pva(1)],
                                11: [lambda: proj_k(3, 1)],
                            }
                    elif pair == 0:
                        groups = GROUPS_MIX_A
                        side = {
                            2: [deferred[0]],
                            3: [deferred[1]],
                            4: [deferred[2]],
                            5: [lambda: pva(0)],
                            6: [lambda: pva(1)],
                            7: [lambda qc=qc: outproj2(qc - 1, 0)],
                            8: [lambda qc=qc: outproj2(qc - 1, 1)],
                            9: [lambda qc=qc: proj_q(qc, 1)],
                        }
                    else:
                        groups = GROUPS_MIX_B
                        side = {
                            2: [deferred[0]],
                            3: [deferred[1]],
                            4: [deferred[2]],
                            5: [lambda: pva(0)],
                            6: [lambda: pva(1)],
                        }
                        side[7] = [lambda qc=qc: outproj2(qc - 1, 2)]
                        side[9] = [lambda qc=qc: outproj2(qc - 1, 3)]
                        if qc < NQC - 1:
                            side[8] = [lambda qc=qc: proj_q(qc + 1, 0)]

                    for gi, (kt2s, tag) in enumerate(groups):
                        scores_group(qc, pair, kt2s, tag, pm)
                        for fn in side.get(gi, ()):
                            fn()

                    def pvb(j, pair=pair, pm=pm, xacc=xacc, xaccF=xaccF):
                        pv_burst(pair, 1, j, pm, xacc, xaccF)

                    def norm(qc=qc, pair=pair, xaccF=xaccF):
                        normalize(qc, pair, xaccF)

                    deferred = [lambda f=pvb: f(0), lambda f=pvb: f(1),
                                lambda f=norm: f()]

            for fn in deferred:
                fn()
            for i in range(4):
                outproj2(NQC - 1, i, evac="scalar")

    _split_waits(nc, mybir)
    return nc


def _get_nc():
    if "nc" not in _NC_CACHE:
        _NC_CACHE["nc"] = _build_nc()
    return _NC_CACHE["nc"]


def _install_profile_hook():
    """Provide antenv.axon_hooks.get_axon_ntff_profile_hook via ctypes into
    libaxon_pjrt.so when the image's antenv package lacks the module."""
    import types
    import ctypes
    import contextlib
    try:
        from antenv.axon_hooks import get_axon_ntff_profile_hook  # noqa: F401
        return
    except ImportError:
        pass
    so_path = "/opt/axon/libaxon_pjrt.so"
    try:
        lib = ctypes.CDLL(so_path)
    except OSError:
        lib = None
    if lib is None or not hasattr(lib, "axon_start_nrt_profile"):
        hook = None
    else:
        lib.axon_start_nrt_profile.argtypes = [
            ctypes.POINTER(ctypes.c_int64), ctypes.c_size_t]
        lib.axon_start_nrt_profile.restype = ctypes.c_int64
        lib.axon_stop_nrt_profile.argtypes = [ctypes.c_char_p]
        lib.axon_stop_nrt_profile.restype = ctypes.c_int64

        @contextlib.contextmanager
        def hook(output_dir, device_ids):
            import jax
            jax.devices()
            if device_ids:
                ids = (ctypes.c_int64 * len(device_ids))(*device_ids)
                rc = lib.axon_start_nrt_profile(ids, len(device_ids))
            else:
                rc = lib.axon_start_nrt_profile(None, 0)
            if rc != 0:
                raise RuntimeError(f"axon_start_nrt_profile rc={rc}")
            try:
                yield
            finally:
                n = lib.axon_stop_nrt_profile(str(output_dir).encode())
                print(f"profile: {n} ntff file(s) -> {output_dir}",
                      file=sys.stderr)

    import antenv
    mod = types.ModuleType("antenv.axon_hooks")
    mod.get_axon_ntff_profile_hook = lambda: hook
    sys.modules["antenv.axon_hooks"] = mod
    antenv.axon_hooks = mod


def _reference_numpy(query, key, value, mask, w_q, b_q, w_k, b_k, w_v, b_v,
                     w_o, b_o):
    B, S_, D = query.shape
    H = 16
    dk = D // H
    NEG = -1000000000.0

    def proj(x, w, b):
        return (x @ w.T + b).reshape(B, S_, H, dk).transpose(0, 2, 1, 3)

    q = proj(query, w_q, b_q)
    k = proj(key, w_k, b_k)
    v = proj(value, w_v, b_v)
    scores = np.einsum("bhqd,bhkd->bhqk", q, k) / np.sqrt(np.float32(dk))
    scores = np.where(mask[:, None, :, :] == 0, NEG, scores)
    scores = scores - scores.max(axis=-1, keepdims=True)
    e = np.exp(scores)
    p = e / e.sum(axis=-1, keepdims=True)
    x = np.einsum("bhqk,bhkd->bhqd", p, v)
    x = x.transpose(0, 2, 1, 3).reshape(B, S_, D)
    return (x @ w_o.T + b_o).astype(np.float32)


def _swizzle_x(x, bf):
    """[S, DM] f32 -> [P, NQC, KT, QC] bf16 (x^T tiled for 8KB descriptors)."""
    t = np.ascontiguousarray(x.T).reshape(KT, P, NQC, QC)
    return np.ascontiguousarray(t.transpose(1, 2, 0, 3)).astype(bf)


def kernel(query, key, value, mask, w_q, b_q, w_k, b_k, w_v, b_v, w_o, b_o):
    global LAST_EXEC_NS, LAST_RESULTS
    import ml_dtypes
    bf = ml_dtypes.bfloat16

    query = np.asarray(query, np.float32)
    key = np.asarray(key, np.float32)
    value = np.asarray(value, np.float32)
    mask_np = np.asarray(mask)
    w_q = np.asarray(w_q, np.float32)
    b_q = np.asarray(b_q, np.float32)
    w_k = np.asarray(w_k, np.float32)
    b_k = np.asarray(b_k, np.float32)
    w_v = np.asarray(w_v, np.float32)
    b_v = np.asarray(b_v, np.float32)
    w_o = np.asarray(w_o, np.float32)
    b_o = np.asarray(b_o, np.float32)

    # Device fast path assumes an all-ones mask and zero qkv biases (true for
    # this problem's setup_inputs); anything else falls back to numpy.
    if (mask_np != 1).any() or b_q.any() or b_k.any() or b_v.any():
        return _reference_numpy(query, key, value, mask_np, w_q, b_q, w_k,
                                b_k, w_v, b_v, w_o, b_o)

    from concourse import bass_utils

    nc = _get_nc()

    xT = {b: {
        "xq": _swizzle_x(query[b], bf),
        "xk": _swizzle_x(key[b], bf),
        "xv": _swizzle_x(value[b], bf),
    } for b in range(2)}

    def _sw_w(w):     # [DH, DM] slice -> [P, KT, DH]
        return np.ascontiguousarray(
            w.T.reshape(KT, P, DH).transpose(1, 0, 2)).astype(bf)

    def _sw_wo(w):    # [DM, DH] slice -> [P, 2, DM]
        return np.ascontiguousarray(
            w.T.reshape(2, P, DM).transpose(1, 0, 2)).astype(bf)

    in_maps = []
    for c in range(N_CORES):
        b = c // 4
        g = c % 4
        fs = slice(DH * g, DH * (g + 1))
        in_maps.append({
            **xT[b],
            "wq": _sw_w(w_q[fs, :]),
            "wk": _sw_w(w_k[fs, :]),
            "wv": _sw_w(w_v[fs, :]),
            "wo": _sw_wo(w_o[:, fs]),
        })

    if PROFILE:
        _install_profile_hook()
    res = bass_utils.run_bass_kernel_spmd(
        nc, in_maps, core_ids=list(range(N_CORES)), trace=PROFILE)
    if PROFILE:
        LAST_EXEC_NS = res.exec_time_ns
        LAST_RESULTS = res

    out = np.empty((2, S, DM), np.float32)
    for b in range(2):
        acc = res.results[4 * b]["outT"].astype(np.float32)
        for g in range(1, 4):
            acc += res.results[4 * b + g]["outT"].astype(np.float32)
        # [P, NQC, 8, QC] -> [DM, S] -> [S, DM]
        full = acc.transpose(2, 0, 1, 3).reshape(DM, S)
        out[b] = full.T
    out += b_o
    return out
